# revision 1
# baseline (speedup 1.0000x reference)
"""DeepGCN (GENConv softmax-aggregation, 4 layers) on 8 Trainium2 NeuronCores.

Strategy (graph/data parallel per sharding hint):
  - Nodes partitioned contiguously across 8 cores (6250 each); edges assigned
    to the core owning their dst node, sorted by dst, padded per 128-node tile
    so every core runs an identical (SPMD) program.
  - Per layer: gather source features r[src] from a replicated DRAM table
    (bf16 [50000,128]) via indirect DMA (32 edge-tiles / 4096 rows per call);
    softmax aggregation is computed with indicator matmuls accumulating
    [denom | num] in PSUM per 128-node tile; the per-node MLP runs
    feature-on-partition; residual h stays resident in SBUF.
  - Between layers, each core's slice of r'=relu(BN(h))+edge_b is AllGathered
    into the next layer's gather table.
  - Graph mean-pool partials ([64,128] per core) are summed on host; the tiny
    136x2 classifier runs on host.

Numerics: gather table, edge elementwise chain and aggregation matmuls in
bf16 (PSUM accumulates fp32); node-phase MLP in fp32. Softmax max-subtraction
is algebraically redundant here (z = t*relu(...) is tiny) and omitted; the
1e-7 message epsilon shifts the output by <=1e-7 and is omitted.
"""

import numpy as np
import ml_dtypes

import concourse.bass as bass
import concourse.bacc as bacc
import concourse.tile as tile
from concourse import mybir
from concourse.masks import make_identity
from concourse.bass_utils import run_bass_kernel_spmd

F32 = mybir.dt.float32
BF16 = mybir.dt.bfloat16
I32 = mybir.dt.int32

N, E, C, H, L, G, K, NCLS = 50000, 500000, 256, 128, 4, 64, 8, 2
NCORES = 8
NPC = N // NCORES          # 6250 nodes per core
NT = (NPC + 127) // 128    # 49 node tiles per core
NPC_PAD = NT * 128         # 6272
Q = 16                     # edge tiles per gather group
EPS_BN = 1e-5
P = 128

_cache = {}
DEBUG = False


def _ap_view(t, extra_offset, pattern):
    base = t[:]
    return bass.AP(base.tensor, base.offset + extra_offset, [base.ap[0]] + pattern)


def _build(TE, ET, t_vals):
    """Build + compile the SPMD program for edge-tile counts ET (len NT)."""
    nt_of = np.repeat(np.arange(NT), ET)
    tile_starts = np.concatenate([[0], np.cumsum(ET)])
    first_of = set(tile_starts[:-1].tolist())
    last_of = set((tile_starts[1:] - 1).tolist())
    NG = (TE + Q - 1) // Q

    nc = bacc.Bacc("TRN2", target_bir_lowering=False, debug=False,
                   num_devices=NCORES)

    # ---- kernel I/O ----
    x_in = nc.dram_tensor("x", [NPC_PAD, C], F32, kind="ExternalInput")
    esrc_in = nc.dram_tensor("esrc", [P, TE], I32, kind="ExternalInput")
    eattr_in = nc.dram_tensor("eattr", [P, TE], F32, kind="ExternalInput")
    edloc_in = nc.dram_tensor("edloc", [P, TE], I32, kind="ExternalInput")
    batch_in = nc.dram_tensor("batch", [P, NT], I32, kind="ExternalInput")
    lsw_in = nc.dram_tensor("lsw", [C, H], F32, kind="ExternalInput")
    ldw_in = nc.dram_tensor("ldw", [C, H], F32, kind="ExternalInput")
    lsb_in = nc.dram_tensor("lsb", [H], F32, kind="ExternalInput")   # + edge_b[0]
    ldb_in = nc.dram_tensor("ldb", [H], F32, kind="ExternalInput")
    ew_in = nc.dram_tensor("ew", [L, H], F32, kind="ExternalInput")
    eb_in = nc.dram_tensor("eb", [L, H], F32, kind="ExternalInput")
    w1_in = nc.dram_tensor("w1f", [L, H, 2 * H], F32, kind="ExternalInput")
    b1_in = nc.dram_tensor("b1f", [L, 2 * H], F32, kind="ExternalInput")
    w2_in = nc.dram_tensor("w2", [L, 2 * H, H], F32, kind="ExternalInput")
    b2_in = nc.dram_tensor("b2", [L, H], F32, kind="ExternalInput")
    bns_in = nc.dram_tensor("bns", [L, H], F32, kind="ExternalInput")
    bnb_in = nc.dram_tensor("bnb", [L, H], F32, kind="ExternalInput")
    pooled_out = nc.dram_tensor("pooled", [G, H], F32, kind="ExternalOutput")
    dbg_outs = [
        nc.dram_tensor(f"dbg{i}", [NPC, H], F32, kind="ExternalOutput")
        for i in range(L)
    ] if DEBUG else []
    if DEBUG:
        dbgfull0 = nc.dram_tensor("dbgfull0", [N, H], F32, kind="ExternalOutput")
        dbgh0 = nc.dram_tensor("dbgh0", [P, NPC_PAD], F32, kind="ExternalOutput")
        dbggx = nc.dram_tensor("dbggx", [P, Q * 128], F32, kind="ExternalOutput")

    with tile.TileContext(nc) as tc:
        with (
            tc.tile_pool(name="persist", bufs=1) as pp,
            tc.tile_pool(name="wl", bufs=1) as wl,
            tc.tile_pool(name="edge", bufs=2) as ep,
            tc.tile_pool(name="node", bufs=3) as npool,
            tc.tile_pool(name="psA", bufs=2, space="PSUM") as psA,
            tc.tile_pool(name="psB", bufs=2, space="PSUM") as psB,
            tc.tile_pool(name="psC", bufs=2, space="PSUM") as psC,
            tc.tile_pool(name="psT", bufs=1, space="PSUM") as psT,
            tc.tile_pool(name="psP", bufs=1, space="PSUM") as psP,
            tc.tile_pool(name="dram", bufs=4, space="DRAM") as dp,
        ):
            # ---------- persistent state ----------
            hT = pp.tile([P, NPC_PAD], F32, tag="hT")        # residual, [H, nodes]
            skipT = pp.tile([P, NPC_PAD], F32, tag="skipT")  # r_l (dst skip), [H, nodes]

            ident = pp.tile([P, P], F32, tag="ident")
            make_identity(nc, ident[:])
            ones1 = pp.tile([1, P], F32, tag="ones1")
            nc.vector.memset(ones1[:], 1.0)

            idx_all = pp.tile([P, TE], I32, tag="idx")
            nc.sync.dma_start(idx_all[:], esrc_in[:])
            attr_f = pp.tile([P, TE], F32, tag="attrf")
            nc.sync.dma_start(attr_f[:], eattr_in[:])
            dloc_i = pp.tile([P, TE], I32, tag="dloci")
            nc.sync.dma_start(dloc_i[:], edloc_in[:])
            dloc_f = pp.tile([P, TE], F32, tag="dlocf")
            nc.vector.tensor_copy(out=dloc_f[:], in_=dloc_i[:])
            batch_i = pp.tile([P, NT], I32, tag="batchi")
            nc.sync.dma_start(batch_i[:], batch_in[:])
            batch_f = pp.tile([P, NT], F32, tag="batchf")
            nc.vector.tensor_copy(out=batch_f[:], in_=batch_i[:])

            iota_i = pp.tile([P, P], I32, tag="iotai")
            nc.gpsimd.iota(iota_i[:], pattern=[[1, 128]], base=0,
                           channel_multiplier=0)
            iota_f = pp.tile([P, P], F32, tag="iotaf")
            nc.vector.tensor_copy(out=iota_f[:], in_=iota_i[:])
            iota_g = pp.tile([P, G], F32, tag="iotag")
            nc.vector.tensor_copy(out=iota_g[:], in_=iota_i[:, :G])

            # projection weights
            lsw0 = pp.tile([P, H], F32, tag="lsw0")
            lsw1 = pp.tile([P, H], F32, tag="lsw1")
            ldw0 = pp.tile([P, H], F32, tag="ldw0")
            ldw1 = pp.tile([P, H], F32, tag="ldw1")
            nc.sync.dma_start(lsw0[:], lsw_in[0:P, :])
            nc.sync.dma_start(lsw1[:], lsw_in[P : 2 * P, :])
            nc.sync.dma_start(ldw0[:], ldw_in[0:P, :])
            nc.sync.dma_start(ldw1[:], ldw_in[P : 2 * P, :])
            ldb_v = pp.tile([P, 1], F32, tag="ldbv")
            nc.sync.dma_start(ldb_v[:], ldb_in[:, None])

            # srcb broadcast [128 nodes, 128 feat] = ones x lsb (lsb has eb[0] folded)
            lsb_row = pp.tile([1, H], F32, tag="lsbrow")
            nc.sync.dma_start(lsb_row[:], lsb_in[None, :])
            bc_ps = psT.tile([P, P], F32, space="PSUM", tag="trps")
            nc.tensor.matmul(out=bc_ps[:], lhsT=ones1[:], rhs=lsb_row[:],
                             start=True, stop=True)
            srcb_bc = pp.tile([P, P], F32, tag="srcbbc")
            nc.vector.tensor_copy(out=srcb_bc[:], in_=bc_ps[:])

            # per-layer broadcast tiles: wbc_l (bf16), ebbc_l (f32, layers 1..3)
            wbc, ebbc = [], {}
            for l in range(L):
                wrow = wl.tile([1, H], F32, tag=f"wrow{l}")
                nc.sync.dma_start(wrow[:], ew_in[l, :][None, :])
                t_ps = psT.tile([P, P], F32, space="PSUM", tag="trps")
                nc.tensor.matmul(out=t_ps[:], lhsT=ones1[:], rhs=wrow[:],
                                 start=True, stop=True)
                wb = wl.tile([P, P], F32, tag=f"wbc{l}")
                nc.vector.tensor_copy(out=wb[:], in_=t_ps[:])
                wbc.append(wb)
                if l >= 1:
                    erow = wl.tile([1, H], F32, tag=f"erow{l}")
                    nc.sync.dma_start(erow[:], eb_in[l, :][None, :])
                    t_ps2 = psT.tile([P, P], F32, space="PSUM", tag="trps")
                    nc.tensor.matmul(out=t_ps2[:], lhsT=ones1[:], rhs=erow[:],
                                     start=True, stop=True)
                    ebb = wl.tile([P, P], F32, tag=f"ebbc{l}")
                    nc.vector.tensor_copy(out=ebb[:], in_=t_ps2[:])
                    ebbc[l] = ebb

            # per-layer MLP / norm params
            w1s, b1a, b1b, w2a, w2b, b2v, bnsv, bnbv = [], [], [], [], [], [], [], []
            for l in range(L):
                w1 = wl.tile([P, 2 * H], F32, tag=f"w1{l}")
                nc.sync.dma_start(w1[:], w1_in[l])
                w1s.append(w1)
                ba = wl.tile([P, 1], F32, tag=f"b1a{l}")
                nc.sync.dma_start(ba[:], b1_in[l, 0:H][:, None])
                b1a.append(ba)
                bb = wl.tile([P, 1], F32, tag=f"b1b{l}")
                nc.sync.dma_start(bb[:], b1_in[l, H : 2 * H][:, None])
                b1b.append(bb)
                wa = wl.tile([P, H], F32, tag=f"w2a{l}")
                nc.sync.dma_start(wa[:], w2_in[l, 0:H, :])
                w2a.append(wa)
                wb2 = wl.tile([P, H], F32, tag=f"w2b{l}")
                nc.sync.dma_start(wb2[:], w2_in[l, H : 2 * H, :])
                w2b.append(wb2)
                bv = wl.tile([P, 1], F32, tag=f"b2{l}")
                nc.sync.dma_start(bv[:], b2_in[l, :][:, None])
                b2v.append(bv)
                sv = wl.tile([P, 1], F32, tag=f"bns{l}")
                nc.sync.dma_start(sv[:], bns_in[l, :][:, None])
                bnsv.append(sv)
                bvv = wl.tile([P, 1], F32, tag=f"bnb{l}")
                nc.sync.dma_start(bvv[:], bnb_in[l, :][:, None])
                bnbv.append(bvv)

            # gather tables (DRAM)
            g_local = [dp.tile([NPC, H], F32, tag="glocal", name=f"glocal{i}")
                       for i in range(L)]
            g_full = [dp.tile([N, H], F32, tag="gfull", name=f"gfull{i}")
                      for i in range(L)]

            # ---------- phase A: layer-0 projections ----------
            for nt in range(NT):
                nb = nt * 128
                rows = min(128, NPC - nb)  # last tile partial for the DMA out
                xt = npool.tile([P, C], F32, tag="xt")
                nc.sync.dma_start(xt[:], x_in[nb : nb + 128, :])
                xT0 = npool.tile([P, P], F32, tag="xT0")
                xT1 = npool.tile([P, P], F32, tag="xT1")
                tp = psT.tile([P, P], F32, space="PSUM", tag="trps")
                nc.tensor.transpose(out=tp[:], in_=xt[:, 0:P], identity=ident[:])
                nc.vector.tensor_copy(out=xT0[:], in_=tp[:])
                tp2 = psT.tile([P, P], F32, space="PSUM", tag="trps")
                nc.tensor.transpose(out=tp2[:], in_=xt[:, P : 2 * P], identity=ident[:])
                nc.vector.tensor_copy(out=xT1[:], in_=tp2[:])

                # xs0 in [node, feat]: lhsT = xT chunk, rhs = lsw chunk
                ps_xs = psB.tile([P, 2 * H], F32, space="PSUM", tag="mlp1")
                nc.tensor.matmul(out=ps_xs[:, 0:H], lhsT=xT0[:], rhs=lsw0[:],
                                 start=True, stop=False)
                nc.tensor.matmul(out=ps_xs[:, 0:H], lhsT=xT1[:], rhs=lsw1[:],
                                 start=False, stop=True)
                rw = npool.tile([P, H], F32, tag="rw")
                nc.vector.tensor_add(out=rw[:], in0=ps_xs[:, 0:H], in1=srcb_bc[:])
                nc.sync.dma_start(g_local[0][nb : nb + rows, :], rw[:rows, :])

                # xd0 in [feat, node]: lhsT = ldw chunk, rhs = xT chunk
                ps_xd = psC.tile([P, H], F32, space="PSUM", tag="mlp2")
                nc.tensor.matmul(out=ps_xd[:], lhsT=ldw0[:], rhs=xT0[:],
                                 start=True, stop=False)
                nc.tensor.matmul(out=ps_xd[:], lhsT=ldw1[:], rhs=xT1[:],
                                 start=False, stop=True)
                nc.scalar.activation(out=skipT[:, nb : nb + 128], in_=ps_xd[:],
                                     func=mybir.ActivationFunctionType.Identity,
                                     bias=ldb_v[:, :1], scale=1.0)

            nc.gpsimd.collective_compute(
                "AllGather", mybir.AluOpType.bypass,
                replica_groups=[list(range(NCORES))],
                ins=[g_local[0].opt()], outs=[g_full[0].opt()],
            )

            # ---------- layers ----------
            pool_ps = None
            for l in range(L):
                if DEBUG and l == 1:
                    nc.sync.dma_start(dbgfull0[:], g_full[0][:])
                    nc.sync.dma_start(dbgh0[:], hT[:])
                ps_agg = {}
                for g in range(NG):
                    j0 = g * Q
                    qw = min(Q, TE - j0)
                    W = qw * 128
                    gx = ep.tile([P, Q * 128], F32, tag="gx", bufs=4)
                    for k in range(qw):
                        nc.gpsimd.indirect_dma_start(
                            out=gx[:, k * 128 : (k + 1) * 128],
                            out_offset=None,
                            in_=g_full[l][:],
                            in_offset=bass.IndirectOffsetOnAxis(
                                ap=idx_all[:, j0 + k : j0 + k + 1], axis=0),
                        )
                    if DEBUG and l == 0 and g == 0:
                        nc.sync.dma_start(dbggx[:], gx[:])
                    # u = gx + attr*w, msg = relu(u)  (f32 arithmetic;
                    # gx is bf16 from the gather table)
                    u = ep.tile([P, Q * 128], F32, tag="u")
                    av = _ap_view(attr_f, j0, [[1, qw], [0, 128]])
                    wv = _ap_view(wbc[l], 0, [[0, qw], [1, 128]])
                    nc.vector.tensor_tensor(out=u[:, 0:W], in0=av, in1=wv,
                                            op=mybir.AluOpType.mult)
                    nc.vector.tensor_add(out=u[:, 0:W], in0=u[:, 0:W],
                                         in1=gx[:, 0:W])
                    nc.vector.tensor_scalar(out=u[:, 0:W], in0=u[:, 0:W],
                                            scalar1=0.0, scalar2=None,
                                            op0=mybir.AluOpType.max)
                    # emz interleaved [ez | msg*ez] per edge tile
                    emz = ep.tile([P, Q * 256], BF16, tag="emz")
                    msg_v = _ap_view(u, 0, [[128, qw], [1, 128]])
                    ez_v = _ap_view(emz, 0, [[256, qw], [1, 128]])
                    mez_v = _ap_view(emz, 128, [[256, qw], [1, 128]])
                    nc.scalar.activation(out=ez_v, in_=msg_v,
                                         func=mybir.ActivationFunctionType.Exp,
                                         scale=float(t_vals[l]))
                    nc.vector.tensor_tensor(out=mez_v, in0=msg_v, in1=ez_v,
                                            op=mybir.AluOpType.mult)
                    # indicator
                    ind = ep.tile([P, Q * 128], BF16, tag="ind")
                    dv = _ap_view(dloc_f, j0, [[1, qw], [0, 128]])
                    iv = _ap_view(iota_f, 0, [[0, qw], [1, 128]])
                    nc.vector.tensor_tensor(out=ind[:, 0:W], in0=dv,
                                            in1=iv,
                                            op=mybir.AluOpType.is_equal)
                    for k in range(qw):
                        j = j0 + k
                        nt = int(nt_of[j])
                        if j in first_of:
                            ps_agg[nt] = psA.tile(
                                [P, 2 * H], F32, space="PSUM", tag="agg",
                                name=f"agg{nt}", bufs=2)
                        nc.tensor.matmul(
                            out=ps_agg[nt][:],
                            lhsT=ind[:, k * 128 : (k + 1) * 128],
                            rhs=emz[:, k * 256 : (k + 1) * 256],
                            start=(j in first_of), stop=(j in last_of),
                        )
                        if j not in last_of:
                            continue
                        # ---------- node phase for nt ----------
                        nb = nt * 128
                        rows = min(128, NPC - nb)
                        pa = ps_agg.pop(nt)
                        dmax = npool.tile([P, H], F32, tag="dmax")
                        nc.vector.tensor_scalar(out=dmax[:], in0=pa[:, 0:H],
                                                scalar1=1e-16, scalar2=None,
                                                op0=mybir.AluOpType.max)
                        drec = npool.tile([P, H], F32, tag="drec")
                        nc.vector.reciprocal(out=drec[:], in_=dmax[:])
                        aggs = npool.tile([P, H], F32, tag="aggs")
                        nc.vector.tensor_mul(out=aggs[:], in0=pa[:, H : 2 * H],
                                             in1=drec[:])
                        tp3 = psT.tile([P, P], F32, space="PSUM", tag="trps")
                        nc.tensor.transpose(out=tp3[:], in_=aggs[:],
                                            identity=ident[:])
                        outT = npool.tile([P, P], F32, tag="outT")
                        nc.vector.tensor_add(out=outT[:], in0=tp3[:],
                                             in1=skipT[:, nb : nb + 128])
                        # MLP
                        pm1 = psB.tile([P, 2 * H], F32, space="PSUM", tag="mlp1")
                        nc.tensor.matmul(out=pm1[:, 0:H], lhsT=w1s[l][:, 0:H],
                                         rhs=outT[:], start=True, stop=True)
                        nc.tensor.matmul(out=pm1[:, H : 2 * H],
                                         lhsT=w1s[l][:, H : 2 * H],
                                         rhs=outT[:], start=True, stop=True)
                        h1a = npool.tile([P, P], F32, tag="h1a")
                        nc.scalar.activation(out=h1a[:], in_=pm1[:, 0:H],
                                             func=mybir.ActivationFunctionType.Relu,
                                             bias=b1a[l][:, :1], scale=1.0)
                        h1b = npool.tile([P, P], F32, tag="h1b")
                        nc.scalar.activation(out=h1b[:], in_=pm1[:, H : 2 * H],
                                             func=mybir.ActivationFunctionType.Relu,
                                             bias=b1b[l][:, :1], scale=1.0)
                        pm2 = psC.tile([P, H], F32, space="PSUM", tag="mlp2")
                        nc.tensor.matmul(out=pm2[:], lhsT=w2a[l][:], rhs=h1a[:],
                                         start=True, stop=False)
                        nc.tensor.matmul(out=pm2[:], lhsT=w2b[l][:], rhs=h1b[:],
                                         start=False, stop=True)
                        hslice = hT[:, nb : nb + 128]
                        if l == 0:
                            nc.scalar.activation(
                                out=hslice, in_=pm2[:],
                                func=mybir.ActivationFunctionType.Identity,
                                bias=b2v[l][:, :1], scale=1.0)
                        else:
                            tmp = npool.tile([P, P], F32, tag="htmp")
                            nc.scalar.activation(
                                out=tmp[:], in_=pm2[:],
                                func=mybir.ActivationFunctionType.Identity,
                                bias=b2v[l][:, :1], scale=1.0)
                            nc.vector.tensor_add(out=hslice, in0=hslice,
                                                 in1=tmp[:])
                        if l < L - 1:
                            # r_{l+1} = relu(bn_{l+1}(h)); also next skip
                            nc.scalar.activation(
                                out=skipT[:, nb : nb + 128], in_=hslice,
                                func=mybir.ActivationFunctionType.Relu,
                                bias=bnbv[l + 1][:, :1], scale=bnsv[l + 1][:, :1])
                            tp4 = psT.tile([P, P], F32, space="PSUM", tag="trps")
                            nc.tensor.transpose(out=tp4[:],
                                                in_=skipT[:, nb : nb + 128],
                                                identity=ident[:])
                            rw2 = npool.tile([P, H], F32, tag="rw")
                            nc.vector.tensor_add(out=rw2[:], in0=tp4[:],
                                                 in1=ebbc[l + 1][:])
                            nc.sync.dma_start(
                                g_local[l + 1][nb : nb + rows, :], rw2[:rows, :])
                        else:
                            # final norm (layer 0 params) + pooling partials
                            fT = npool.tile([P, P], F32, tag="fT")
                            nc.scalar.activation(
                                out=fT[:], in_=hslice,
                                func=mybir.ActivationFunctionType.Relu,
                                bias=bnbv[0][:, :1], scale=bnsv[0][:, :1])
                            tp5 = psT.tile([P, P], F32, space="PSUM", tag="trps")
                            nc.tensor.transpose(out=tp5[:], in_=fT[:],
                                                identity=ident[:])
                            fr = npool.tile([P, P], F32, tag="fr")
                            nc.vector.tensor_copy(out=fr[:], in_=tp5[:])
                            gind = npool.tile([P, G], F32, tag="gind")
                            bv2 = _ap_view(batch_f, nt, [[1, 1], [0, G]])
                            nc.vector.tensor_tensor(out=gind[:], in0=bv2,
                                                    in1=iota_g[:],
                                                    op=mybir.AluOpType.is_equal)
                            if pool_ps is None:
                                pool_ps = psP.tile([G, H], F32, space="PSUM",
                                                   tag="pool")
                            nc.tensor.matmul(out=pool_ps[:], lhsT=gind[:, 0:G],
                                             rhs=fr[:], start=(nt == 0),
                                             stop=(nt == NT - 1))
                if l < L - 1:
                    nc.gpsimd.collective_compute(
                        "AllGather", mybir.AluOpType.bypass,
                        replica_groups=[list(range(NCORES))],
                        ins=[g_local[l + 1].opt()], outs=[g_full[l + 1].opt()],
                    )

            for i in range(L):
                if DEBUG:
                    nc.sync.dma_start(dbg_outs[i][:], g_local[i][:])
            pool_s = pp.tile([G, H], F32, tag="pools")
            nc.vector.tensor_copy(out=pool_s[:], in_=pool_ps[:])
            nc.sync.dma_start(pooled_out[:], pool_s[:])

    nc.compile()
    return nc


def _prep(edge_index, edge_attr):
    src = edge_index[0].astype(np.int64)
    dst = edge_index[1].astype(np.int64)
    core = dst // NPC
    tloc = (dst % NPC) // 128
    cnt = np.zeros((NCORES, NT), np.int64)
    np.add.at(cnt, (core, tloc), 1)
    ET = np.maximum(np.ceil(cnt / 128.0).astype(np.int64).max(axis=0), 1)
    TE = int(ET.sum())
    starts = (np.concatenate([[0], np.cumsum(ET)]) * 128).astype(np.int64)

    order = np.lexsort((tloc, core))
    sc, st = core[order], tloc[order]
    ssrc = src[order]
    sdst = dst[order]
    sattr = edge_attr.reshape(-1)[order].astype(np.float32)
    gid = sc * NT + st
    counts_flat = np.bincount(gid, minlength=NCORES * NT)
    offs = np.concatenate([[0], np.cumsum(counts_flat)])[:-1]
    rank = np.arange(E) - offs[gid]
    pos = starts[st] + rank

    esrc = np.zeros((NCORES, TE * 128), np.int32)
    eattr = np.zeros((NCORES, TE * 128), np.float32)
    edloc = np.full((NCORES, TE * 128), -1, np.int32)
    esrc[sc, pos] = ssrc.astype(np.int32)
    eattr[sc, pos] = sattr
    edloc[sc, pos] = ((sdst % NPC) - st * 128).astype(np.int32)

    # [core, TE*128] -> [core, 128, TE]  (edge (tile j, partition p) at j*128+p)
    esrc_T = np.ascontiguousarray(esrc.reshape(NCORES, TE, 128).transpose(0, 2, 1))
    eattr_T = np.ascontiguousarray(eattr.reshape(NCORES, TE, 128).transpose(0, 2, 1))
    edloc_T = np.ascontiguousarray(edloc.reshape(NCORES, TE, 128).transpose(0, 2, 1))
    return TE, ET, esrc_T, eattr_T, edloc_T


def kernel(x, edge_index, edge_attr, batch, clinical,
           lin_src_w, lin_src_b, lin_dst_w, lin_dst_b,
           edge_w, edge_b, t,
           mlp_w1, mlp_b1, mlp_bn_g, mlp_bn_b, mlp_bn_m, mlp_bn_v,
           mlp_w2, mlp_b2, norm_g, norm_b, norm_m, norm_v,
           cls_w, cls_b):
    x = np.asarray(x, np.float32)
    edge_index = np.asarray(edge_index)
    edge_attr = np.asarray(edge_attr, np.float32)
    batch = np.asarray(batch)
    t = np.asarray(t, np.float32)

    TE, ET, esrc_T, eattr_T, edloc_T = _prep(edge_index, edge_attr)

    key = (TE, tuple(int(v) for v in ET), t.tobytes())
    if key not in _cache:
        _cache.clear()
        _cache[key] = _build(TE, ET, [float(v) for v in t])
    nc = _cache[key]

    # folded params (host)
    norm_g = np.asarray(norm_g, np.float32)
    norm_v = np.asarray(norm_v, np.float32)
    s_bn = norm_g / np.sqrt(norm_v + EPS_BN)
    b_bn = np.asarray(norm_b, np.float32) - np.asarray(norm_m, np.float32) * s_bn
    s1 = np.asarray(mlp_bn_g, np.float32) / np.sqrt(np.asarray(mlp_bn_v, np.float32) + EPS_BN)
    w1f = np.asarray(mlp_w1, np.float32) * s1[:, None, :]
    b1f = s1 * np.asarray(mlp_b1, np.float32) + (
        np.asarray(mlp_bn_b, np.float32) - np.asarray(mlp_bn_m, np.float32) * s1)
    ew = np.ascontiguousarray(np.asarray(edge_w, np.float32)[:, 0, :])
    eb = np.asarray(edge_b, np.float32)
    lsb_fold = np.asarray(lin_src_b, np.float32) + eb[0]

    # per-core node data
    x_pad = np.zeros((NCORES, NPC_PAD, C), np.float32)
    batch_T = np.full((NCORES, NPC_PAD), -1, np.int32)
    for c in range(NCORES):
        x_pad[c, :NPC] = x[c * NPC : (c + 1) * NPC]
        batch_T[c, :NPC] = batch[c * NPC : (c + 1) * NPC]
    batch_T = np.ascontiguousarray(
        batch_T.reshape(NCORES, NT, 128).transpose(0, 2, 1))

    shared = dict(
        lsw=np.ascontiguousarray(np.asarray(lin_src_w, np.float32)),
        ldw=np.ascontiguousarray(np.asarray(lin_dst_w, np.float32)),
        lsb=lsb_fold, ldb=np.asarray(lin_dst_b, np.float32),
        ew=ew, eb=eb,
        w1f=np.ascontiguousarray(w1f), b1f=np.ascontiguousarray(b1f),
        w2=np.ascontiguousarray(np.asarray(mlp_w2, np.float32)),
        b2=np.asarray(mlp_b2, np.float32),
        bns=np.ascontiguousarray(s_bn), bnb=np.ascontiguousarray(b_bn),
    )
    in_maps = [
        dict(shared, x=x_pad[c], esrc=esrc_T[c], eattr=eattr_T[c],
             edloc=edloc_T[c], batch=batch_T[c])
        for c in range(NCORES)
    ]

    res = run_bass_kernel_spmd(nc, in_maps, core_ids=list(range(NCORES)))
    kernel.last = (nc, in_maps)

    pooled = np.zeros((G, H), np.float64)
    for c in range(NCORES):
        pooled += res.results[c]["pooled"].astype(np.float64)
    cnt = np.bincount(np.asarray(batch), minlength=G).astype(np.float64)
    pooled = (pooled / np.maximum(cnt, 1.0)[:, None]).astype(np.float32)
    z = np.concatenate([pooled, np.asarray(clinical, np.float32)], axis=1)
    return z @ np.asarray(cls_w, np.float32) + np.asarray(cls_b, np.float32)

